# revision 6
# baseline (speedup 1.0000x reference)
"""Contrastive diversity loss (masked logsumexp over the 8192x8192 cosine
similarity matrix) on 8 Trainium2 NeuronCores.

Strategy (v2)
-------------
x (8,128,16,8,8) -> feats [N=8192, F=128]; rows L2-normalized on host and
quantized to fp8e4m3.  Device holds xnT = x_norm.T [F=128, N=8192] fp8,
rotated by 512*core so a single SPMD program covers all unordered group
pairs exactly once (same 17-cell symmetry scheme as v1, now flattened to
66 independent [128,512] blocks packed into 22 psum tiles of [128,1536]).

Per tile the PE runs fp8 DoubleRow matmuls (k-tile dim duplicated via a
stride-0 broadcast; results are 2x the dot product, absorbed into the
activation scale).  DoubleRow streams 2 output cols/cycle, and LDWEIGHTS
overlaps with the previous matmul, so a 512-col block costs 256 PE cycles.

Elementwise exp is split across two engines:
 - ScalarE tiles: ACTIVATE(Exp, scale=5, bias=-10) with accum_out giving
   the per-partition row sums directly (out dumped to a bf16 scratch).
 - DVE tiles: one tensor_scalar computes i16 = 5*log2e*128*g2 + B, the
   Schraudolph bit-trick: the int16 bit pattern *is* bf16(exp(10g-10))
   to ~+-3% with mean ~0 (B embeds the mean-zero correction C=0.045).
   TensorE then row-sums the bf16 view via ones-matmuls (128-col chunks
   as LDWEIGHTS, 1-col matmul each) accumulated into one PSUM bank.

The 2 diagonal cells run on ScalarE: exp of the 4 diagonal [128x128]
blocks lands in a bf16 tile that is DMA'd out so the host can subtract
the true diagonal exactly as the device computed it.

PSUM: 2 x [128,1536] main tiles (6 banks) + reduce accumulator (1 bank)
+ junk bank for the PE warm-up/keep-warm fillers (the PE p-state needs
~3us of continuous work to reach 2.4 GHz).

Host: A = sum(off-diag cols) + PE-acc, D = diag cols;
      total = 2A + D - diag; loss = 10 + log(total).
"""

import numpy as np
import ml_dtypes
from contextlib import ExitStack

import concourse.bass as bass
import concourse.tile as tile
from concourse import bacc, mybir
from concourse.bass_utils import run_bass_kernel_spmd

N = 8192
F = 128
GW = 512            # column-group width
NG = N // GW        # 16 groups
NCORES = 8
TW = 1536           # psum tile width (3 blocks)
LOG2E = 1.4426950408889634
SCH_C = 0.045       # Schraudolph mean-zero correction
# psum value = 2*g (DoubleRow duplicate); want exp(10g - 10)
ACT_SCALE = 5.0
S1_CONST = 5.0 * LOG2E * 128.0                       # tensor_scalar mult
S2_CONST = (127.0 - 10.0 * LOG2E - SCH_C) * 128.0    # tensor_scalar add

# ---- static work list ----------------------------------------------------
# off-diagonal cells (slot pairs), same coverage proof as v1
OFF_CELLS = [(0, d) for d in range(1, 9)] + [(8, 8 + d) for d in range(1, 8)]
# blocks: (weight_col_start, rhs_col_start, width)
OFF_BLOCKS = []
for rho, kap in OFF_CELLS:
    for b in range(4):
        OFF_BLOCKS.append((rho * GW + b * 128, kap * GW, GW))
assert len(OFF_BLOCKS) == 60

N_OFF_TILES = 20                      # 60 blocks / 3 per tile
N_TILES = N_OFF_TILES + 2             # + 2 diagonal-cell tiles
DIAG_SLOTS = (0, 8)

# tile schedule: engine per tile. 9 off-diag ScE + 2 diag ScE + 11 DVE,
# interleaved so both consumers stay fed. Diag tiles at positions 5, 15.
TILE_PLAN = []  # list of ("S"|"D"|"DIAG", payload)
_off_iter = iter(range(N_OFF_TILES))
_pat = ["S", "D", "S", "D", "S", "DIAG", "D", "S", "D", "S", "D", "S",
        "D", "S", "D", "DIAG", "S", "D", "D", "D", "S", "D"]
assert len(_pat) == N_TILES
assert _pat.count("S") == 9 and _pat.count("D") == 11 and _pat.count("DIAG") == 2

_nc_cache = None


def build_nc():
    f32 = mybir.dt.float32
    bf16 = mybir.dt.bfloat16
    i16 = mybir.dt.int16
    f8 = mybir.dt.float8e4

    nc = bacc.Bacc("TRN2", target_bir_lowering=False, debug=False,
                   num_devices=NCORES)
    xn = nc.dram_tensor("xn", [F, N], f8, kind="ExternalInput")
    out = nc.dram_tensor("out", [128, 64], f32, kind="ExternalOutput")
    outd = nc.dram_tensor("outd", [128, 1024], bf16, kind="ExternalOutput")

    with tile.TileContext(nc) as tc:
        with ExitStack() as ctx:
            rhs_pool = ctx.enter_context(tc.tile_pool(name="rhs", bufs=1))
            ps_pool = ctx.enter_context(
                tc.tile_pool(name="ps", bufs=2, space="PSUM"))
            acc_pool = ctx.enter_context(
                tc.tile_pool(name="acc", bufs=1, space="PSUM"))
            junk_pool = ctx.enter_context(
                tc.tile_pool(name="junk", bufs=1, space="PSUM"))
            scr16_pool = ctx.enter_context(tc.tile_pool(name="scr16", bufs=3))
            scrb_pool = ctx.enter_context(tc.tile_pool(name="scrb", bufs=2))
            misc = ctx.enter_context(tc.tile_pool(name="misc", bufs=1))

            rhs = rhs_pool.tile([F, N], f8)

            acc = acc_pool.tile([128, 512], f32)
            junk = junk_pool.tile([128, 512], f32)

            def dr(ap2, n):
                """stride-0 duplicated k-tile view for DoubleRow"""
                return ap2.unsqueeze(1).broadcast_to([128, 2, n])

            def filler(width=504, slot=0):
                # warm-up fillers read slot 15 (delays only the last input
                # chunk); steady-state fillers read slot 0 (always resident)
                nc.tensor.matmul(
                    junk[:, 0:width],
                    dr(rhs[:, slot * GW:slot * GW + 128], 128),
                    dr(rhs[:, slot * GW:slot * GW + width], width),
                    start=True, stop=True, skip_group_check=True,
                    perf_mode=mybir.MatmulPerfMode.DoubleRow)

            # --- PE warm-up burst: ~14 fillers (~3us at mid clock) with no
            # DMA dependency beyond the WAR edge on the last input chunk.
            for _ in range(14):
                filler(slot=15)

            # --- input DMA, slot 0 first for fast availability
            dma_chunks = [(0, 1), (1, 3), (4, 4), (8, 4), (12, 4)]
            for s0, ns in dma_chunks:
                nc.sync.dma_start(rhs[:, s0 * GW:(s0 + ns) * GW],
                                  xn.ap()[:, s0 * GW:(s0 + ns) * GW])

            bias_t = misc.tile([128, 1], f32)
            nc.vector.memset(bias_t[:], -10.0)
            ones_t = misc.tile([128, 1], bf16)
            nc.vector.memset(ones_t[:], 1.0)
            sums = misc.tile([128, 64], f32)
            nc.gpsimd.memset(sums[:], 0.0)
            diag_out = misc.tile([128, 1024], bf16)

            off_blocks = list(OFF_BLOCKS)
            bpos = 0
            pending_reduce = []   # (scratch_bf_ap, width)
            col = 0
            col_classes = {}      # col -> "A" | "D"
            diag_idx = 0
            n_chunks_total = _pat.count("D") * (TW // 128)
            chunk_no = [0]

            def emit_pe_reduce():
                if not pending_reduce:
                    return
                sap, w = pending_reduce.pop(0)
                nchunk = w // 128
                for c in range(nchunk):
                    chunk_no[0] += 1
                    nc.tensor.matmul(
                        acc[:, 0:1], sap[:, c * 128:(c + 1) * 128], ones_t[:],
                        start=(chunk_no[0] == 1),
                        stop=(chunk_no[0] == n_chunks_total),
                        skip_group_check=True)

            for tpos, kind in enumerate(_pat):
                ps = ps_pool.tile([128, TW], f32, tag="ps")
                if kind in ("S", "D"):
                    for k in range(3):
                        wcol, rcol, w = off_blocks[bpos]
                        bpos += 1
                        nc.tensor.matmul(
                            ps[:, k * GW:(k + 1) * GW],
                            dr(rhs[:, wcol:wcol + 128], 128),
                            dr(rhs[:, rcol:rcol + w], w),
                            start=True, stop=True,
                            perf_mode=mybir.MatmulPerfMode.DoubleRow)
                else:
                    # diagonal cell for slot `DIAG_SLOTS[diag_idx]`
                    # layout: [4 diag blocks: 0..512) | uppers: 512..1280)
                    sl = DIAG_SLOTS[diag_idx] * GW
                    up_off = 512
                    for b in range(4):
                        nc.tensor.matmul(
                            ps[:, b * 128:(b + 1) * 128],
                            dr(rhs[:, sl + b * 128:sl + (b + 1) * 128], 128),
                            dr(rhs[:, sl + b * 128:sl + (b + 1) * 128], 128),
                            start=True, stop=True,
                            perf_mode=mybir.MatmulPerfMode.DoubleRow)
                        if b < 3:
                            w = (3 - b) * 128
                            nc.tensor.matmul(
                                ps[:, up_off:up_off + w],
                                dr(rhs[:, sl + b * 128:sl + (b + 1) * 128], 128),
                                dr(rhs[:, sl + (b + 1) * 128:sl + 512], w),
                                start=True, stop=True,
                                perf_mode=mybir.MatmulPerfMode.DoubleRow)
                            up_off += w

                # interleave: PE-reduce for the previous DVE tile, and a
                # keep-warm filler, queued behind this tile's mains.
                emit_pe_reduce()
                filler()

                if kind == "S":
                    exb = scrb_pool.tile([128, TW], bf16, tag="exb")
                    nc.scalar.activation(
                        out=exb[:], in_=ps[:],
                        func=mybir.ActivationFunctionType.Exp,
                        bias=bias_t[:], scale=ACT_SCALE,
                        accum_out=sums[:, col:col + 1])
                    col_classes[col] = "A"
                    col += 1
                elif kind == "D":
                    s16 = scr16_pool.tile([128, TW], i16, tag="s16")
                    nc.vector.tensor_scalar(
                        out=s16[:], in0=ps[:],
                        scalar1=S1_CONST, scalar2=S2_CONST,
                        op0=mybir.AluOpType.mult, op1=mybir.AluOpType.add)
                    pending_reduce.append((s16[:].bitcast(mybir.dt.bfloat16), TW))
                else:
                    # diag tile: ACT#1 over diag blocks [0:512) -> D class,
                    # out into diag_out slice (DMA'd); ACT#2 uppers -> A.
                    dslice = diag_out[:, diag_idx * 512:(diag_idx + 1) * 512]
                    nc.scalar.activation(
                        out=dslice, in_=ps[:, 0:512],
                        func=mybir.ActivationFunctionType.Exp,
                        bias=bias_t[:], scale=ACT_SCALE,
                        accum_out=sums[:, col:col + 1])
                    col_classes[col] = "D"
                    col += 1
                    exb = scrb_pool.tile([128, TW], bf16, tag="exb")
                    nc.scalar.activation(
                        out=exb[:, 0:768], in_=ps[:, 512:1280],
                        func=mybir.ActivationFunctionType.Exp,
                        bias=bias_t[:], scale=ACT_SCALE,
                        accum_out=sums[:, col:col + 1])
                    col_classes[col] = "A"
                    col += 1
                    diag_idx += 1

            # drain remaining PE reduces
            while pending_reduce:
                emit_pe_reduce()
            # close the accumulation group with a final 0-effect... just copy
            nc.vector.tensor_copy(sums[:, 63:64], acc[:, 0:1])
            col_classes[63] = "A"

            nc.sync.dma_start(outd.ap(), diag_out[:])
            nc.sync.dma_start(out.ap(), sums[:])

    nc.compile()
    return nc, dict(col_classes)


def get_nc():
    global _nc_cache
    if _nc_cache is None:
        _nc_cache = build_nc()
    return _nc_cache


def prep_inputs(x):
    """x (8,128,16,8,8) fp32 -> per-core rotated xnT in fp8e4m3."""
    xT = np.ascontiguousarray(
        np.transpose(np.asarray(x, dtype=np.float32), (1, 0, 2, 3, 4))
    ).reshape(F, N)
    norms = np.sqrt((xT ** 2).sum(axis=0, dtype=np.float32))
    norms = np.maximum(norms, np.float32(1e-12)).astype(np.float32)
    xn8 = (xT / norms[None, :]).astype(ml_dtypes.float8_e4m3)
    in_maps = []
    for c in range(NCORES):
        in_maps.append(
            {"xn": np.ascontiguousarray(np.roll(xn8, -GW * c, axis=1))})
    return in_maps


def combine(results, col_classes):
    A = 0.0
    D = 0.0
    dline = 0.0
    for r in results:
        s = r["out"].astype(np.float64).sum(axis=0)
        for c, cls in col_classes.items():
            if cls == "A":
                A += s[c]
            else:
                D += s[c]
        blocks = r["outd"].astype(np.float64).reshape(128, 8, 128)
        for q in range(8):
            dline += np.trace(blocks[:, q, :])
    total = 2.0 * A + D - dline
    return np.float32(10.0 + np.log(total))


def run(x, trace=False, tmpdir=None):
    nc, col_classes = get_nc()
    in_maps = prep_inputs(x)
    res = run_bass_kernel_spmd(nc, in_maps, core_ids=list(range(NCORES)),
                               trace=trace, tmpdir=tmpdir)
    return combine(res.results, col_classes), res


def kernel(x):
    loss, _ = run(x)
    return loss
